# revision 3
# baseline (speedup 1.0000x reference)
"""Trainium2 Bass kernel for nn_BasicS2ConvV2.

Computes out[b,d,p,r] = sum_{c,k,a} W_eff[d,c,k,a,r] * x[b,c,k,p,a], where
W_eff[d,c,k,a,r] = W[d, c, M_idx[k,a,r]] is a pure index-gather of the small
parameter tensor W (materialized on the host).

Device strategy (per NeuronCore, x sharded over p into 8 slices of 1024):
  - The einsum is a matmul with contraction (c,k,a)=4992 = 39 K-tiles of 128.
    M packs (rsub, d) = 4 r's x 32 d's = 128 output partitions; 3 r-groups
    cover r=12.  The moving free dim is p (PT=512 = one fp32 PSUM bank).
  - x is pre-packed ON HOST to bf16 in the exact SBUF tile layout
    xp[b, i, q, t, p'] (t = 39 contraction tiles, q = partition row), so the
    device does pure DMA -> matmul -> out with zero de-stride copies.  The
    ck remainder (rows 384:416) is host-packed: 4 a-slices stacked into one
    K=128 tile (tiles t=36..38), so all matmuls contract a full K=128.
  - Per (b, p-half) unit: x arrives in 4 chunked DMAs (~1.3MB each); the
    3 r-group PSUM banks accumulate in parallel, interleaved chunk-by-chunk,
    so the PE starts after the first chunk and never waits for the tail of
    the unit's load.  x DMAs ride the sync HWDGE queue, W/out the scalar
    queue.
  - Weights are host-packed bf16 lhsT tiles wef[q, rg, t, m], resident in
    SBUF (~3.8MB).
  - Output is written as out[b, rg, m=(rsub*32+d), p] fp32; the host
    transposes to [b, d, p, r] and concatenates the p-shards.
"""

import numpy as np
import ml_dtypes

# Problem shapes (hardcoded; harness runs kernel.py standalone).
B = 2
DIN = 32
DOUT = 32
KK = 13          # kernel size
A = 12           # anchor size
R = 12           # rotation copies
N_PARAM = 36
P_FULL = 8192
N_CORES = 8
P_LOC = P_FULL // N_CORES       # 1024 points per core
CK = DIN * KK                   # 416 contraction rows per a
PT = 512                        # p tile (= 512 fp32 PSUM bank, max moving)
RG = 3                          # r groups (4 r's each)
RSUB = 4
NT = 39                         # lhsT tiles per r-group: 12a x 3ch + 3 packed
NPT = P_LOC // PT               # 2 p tiles per core
CHUNKS = (10, 10, 10, 9)        # x DMA chunking over t

_NC_CACHE = None


def _build_nc(pt=PT, repeat=1):
    import concourse.bacc as bacc
    import concourse.mybir as mybir
    import concourse.tile as tile

    cdt = mybir.dt.bfloat16

    nc = bacc.Bacc("TRN2", target_bir_lowering=False, debug=False,
                   num_devices=N_CORES)
    xp_in = nc.dram_tensor("xp", [B, NPT, 128, NT, pt], cdt,
                           kind="ExternalInput")
    wef_in = nc.dram_tensor("wef", [128, RG, NT, 128], cdt,
                            kind="ExternalInput")
    out_t = nc.dram_tensor("out", [B, RG, 128, P_LOC], mybir.dt.float32,
                           kind="ExternalOutput")

    with tile.TileContext(nc) as tc:
        with (
            tc.tile_pool(name="wpool", bufs=1) as wpool,
            tc.tile_pool(name="xpool", bufs=2) as xpool,
            tc.tile_pool(name="opool", bufs=3) as opool,
            tc.tile_pool(name="pspool", bufs=2, space="PSUM") as pspool,
        ):
          for _rep in range(repeat):
            W_sb = wpool.tile([128, RG, NT, 128], cdt, tag="wsb")
            for rg in range(RG):
                nc.scalar.dma_start(W_sb[:, rg], wef_in[:, rg])

            for b in range(B):
                for i in range(NPT):
                    xt = xpool.tile([128, NT, pt], cdt, tag="x")
                    bnds = []
                    t0 = 0
                    for csz in CHUNKS:
                        nc.sync.dma_start(xt[:, t0:t0 + csz, :],
                                          xp_in[b, i, :, t0:t0 + csz, :])
                        bnds.append((t0, t0 + csz))
                        t0 += csz
                    ps = [pspool.tile([128, pt], mybir.dt.float32,
                                      tag=f"ps{rg}", name=f"ps{rg}")
                          for rg in range(RG)]
                    for (c0, c1) in bnds:
                        for rg in range(RG):
                            for t in range(c0, c1):
                                nc.tensor.matmul(
                                    ps[rg][:, :],
                                    W_sb[:, rg, t, :],
                                    xt[:, t, :],
                                    start=(t == 0), stop=(t == NT - 1))
                    for rg in range(RG):
                        ot = opool.tile([128, pt], mybir.dt.float32,
                                        tag="ot")
                        nc.any.tensor_copy(ot[:], ps[rg][:])
                        nc.scalar.dma_start(
                            out_t[b, rg, :, i * pt:(i + 1) * pt],
                            ot[:])

    nc.compile()
    return nc


def _get_nc():
    global _NC_CACHE
    if _NC_CACHE is None:
        _NC_CACHE = _build_nc()
    return _NC_CACHE


def _host_weights(W, idx_map, idxs_k, idxs_a):
    """Build bf16 lhsT pack wef[q, rg, t, m=(rsub*32+d)].

    Tiles t per r-group: t = a*3+ch (ch<3, rows q = ck=ch*128+q) for the
    full ck chunks; t = 36+j for the packed remainder, whose row q = 32g+qq
    holds ck = 384+qq at a = 4j+g.
    """
    W = np.asarray(W, dtype=np.float32)
    idx_map = np.asarray(idx_map).astype(np.int64)
    idxs_k = np.asarray(idxs_k).astype(np.int64)
    idxs_a = np.asarray(idxs_a).astype(np.int64)

    Wr = W[:, :, idx_map].reshape(DOUT, DIN, KK, A)          # [d,c,k,a]
    a2 = idxs_a                                              # [K,A,R]
    k_ix = np.arange(KK)[:, None, None]
    r_ix = np.arange(R)[None, None, :]
    k2 = idxs_k[k_ix, a2, r_ix]                              # [K,A,R]
    W_eff = Wr[:, :, k2, a2]                                 # [d,c,K,A,R]

    # -> [ck, a, rg, m] with ck = c*13 + k, m = rsub*32 + d, r = rg*4+rsub
    Wf = np.ascontiguousarray(W_eff.transpose(1, 2, 3, 4, 0)).reshape(
        CK, A, R, DOUT).reshape(CK, A, RG, RSUB * DOUT)

    wefA = Wf[:384].reshape(3, 128, A, RG, 128)              # [ch,q,a,rg,m]
    wefA = wefA.transpose(1, 3, 2, 0, 4).reshape(128, RG, 36, 128)

    wefB = Wf[384:].reshape(32, 3, 4, RG, 128)               # [qq,j,g,rg,m]
    wefB = wefB.transpose(2, 0, 3, 1, 4).reshape(128, RG, 3, 128)

    wef = np.concatenate([wefA, wefB], axis=2)               # [128,RG,39,128]
    return np.ascontiguousarray(wef).astype(ml_dtypes.bfloat16)


def _pack_x(x):
    """x [B,DIN,KK,P,A] fp32 -> per-core xp [B,NPT,128,NT,PT] bf16."""
    xbf = np.asarray(x, dtype=np.float32).astype(ml_dtypes.bfloat16)
    xr = xbf.reshape(B, CK, P_FULL, A)
    packs = []
    for core in range(N_CORES):
        xs = xr[:, :, core * P_LOC:(core + 1) * P_LOC, :]
        # full ck chunks: tiles t = a*3+ch, rows q
        xA = xs[:, :384].reshape(B, 3, 128, NPT, PT, A)      # [b,ch,q,i,p,a]
        xA = xA.transpose(0, 3, 2, 5, 1, 4).reshape(B, NPT, 128, 36, PT)
        # remainder: tiles t=36+j, rows q = 32g+qq at a=4j+g
        xB = xs[:, 384:].reshape(B, 32, NPT, PT, 3, 4)       # [b,qq,i,p,j,g]
        xB = xB.transpose(0, 2, 5, 1, 4, 3).reshape(B, NPT, 128, 3, PT)
        packs.append(np.ascontiguousarray(
            np.concatenate([xA, xB], axis=3)))
    return packs


def _prepare_in_maps(inputs):
    wef = _host_weights(inputs["W"], inputs["idx_map"],
                        inputs["idxs_k"], inputs["idxs_a"])
    packs = _pack_x(inputs["x"])
    return [{"xp": packs[core], "wef": wef} for core in range(N_CORES)]


def _decode_out(core_outs):
    """core_outs: list of per-core 'out' arrays [B,RG,128,P_LOC] -> full."""
    shards = []
    for od in core_outs:
        od = np.asarray(od).reshape(B, RG, RSUB, DOUT, P_LOC)
        od = od.transpose(0, 3, 4, 1, 2).reshape(B, DOUT, P_LOC, R)
        shards.append(od)
    return np.ascontiguousarray(np.concatenate(shards, axis=2))


def _run(inputs, trace=False):
    from concourse.bass_utils import run_bass_kernel_spmd

    in_maps = _prepare_in_maps(inputs)
    nc = _get_nc()
    res = run_bass_kernel_spmd(nc, in_maps, core_ids=list(range(N_CORES)),
                               trace=trace)
    out = _decode_out([res.results[c]["out"] for c in range(N_CORES)])
    return out, res


def kernel(**inputs):
    out, _ = _run(inputs, trace=False)
    return out
